# revision 1
# baseline (speedup 1.0000x reference)
"""ArcFace combined-margin loss kernel for 8 TRN2 NeuronCores.

Strategy
--------
reference: cos = (f @ w.T) / (|f||w|); phi = arcface(cos);
outputs = s*(labels*phi + (1-labels)*cos); loss = mean over rows of
-(sum of log_softmax(outputs) at lab_pinds, masked) / L^2.

labels is the multi-hot of (lab_pinds, lengths), so outputs differs from
s*cos only at <=8 entries/row.  Device work is therefore:
  1. C-sharded (2500 classes/core, zero-padded to 2560) dense part: each
     core computes, for all 2048 rows, partial sums
     sexp[b] = sum_c exp(30*cos[b,c] - 30) over its class shard.
     Unit-normalized w rows and raw f rows are transposed on the
     TensorEngine (bf16), evicted from PSUM as fp8(e4m3) scaled by 16,
     and the dots run as fp8 DoubleRow matmuls (K=256 per instruction)
     into bank-aligned PSUM pairs.  PSUM is drained to an SBUF strip and
     one ACT exp per row-block applies scale 30/(256*|f_b|), bias -30,
     with a free-dim accumulate producing the row partial sums.
  2. B-sharded (256 rows/core) positive part: indirect-DMA gather of the
     2048 w rows addressed by lab_pinds, raw fp32 dots with f rows on DVE
     (the positives feed the loss directly, so they stay fp32).
  3. Per-row norm reciprocals (30/|f_b| and 1/|w_c|) as side outputs.
Host (numpy, float64) combines the tiny per-core partials: assembles
cos at positives, applies the arcface margin, corrects the denominator
(exp(30*phi)-exp(30*cos) at positives), logsumexp, masked ragged CE, mean.
No collectives are needed (the only cross-core reduction is over [2048]
scalars, done on host during unsharding).
"""

import math
import sys

import numpy as np

for _p in ("/opt/trn_rl_repo",):
    if _p not in sys.path:
        sys.path.append(_p)

import concourse.bass as bass
import concourse.bacc as bacc
import concourse.mybir as mybir
import concourse.tile as tile
from concourse.bass_utils import run_bass_kernel_spmd
from concourse.masks import make_identity
from contextlib import ExitStack

B, C, D, LMAX = 2048, 20000, 512, 8
NCORES = 8
CSH = C // NCORES          # 2500 real classes per core
CSHP = 2560                # padded to 5*512 (bank-aligned chunks)
BSH = B // NCORES          # 256 rows per core (positives shard)
NBLK = B // 128            # 16 row blocks
NW = 512                   # matmul N-chunk width (exactly one PSUM bank)
NCH = CSHP // NW           # 5 chunks per core
KC = D // 128              # 4 contraction chunks
CT = CSHP // 128           # 20 class tiles (all full)
S = 30.0
M_MARGIN = 0.5

F32 = mybir.dt.float32
BF16 = mybir.dt.bfloat16
FP8 = mybir.dt.float8e4
F8S = 16.0                 # fp8 pre-scale per operand (dots carry 256x)

_GRAPH = None


def build_graph():
    nc = bacc.Bacc()
    f_ext = nc.declare_dram_parameter("f", [B, D], F32, isOutput=False)
    wsh_ext = nc.declare_dram_parameter("wsh", [CSHP, D], F32, isOutput=False)
    w_ext = nc.declare_dram_parameter("w", [C, D], F32, isOutput=False)
    fsh_ext = nc.declare_dram_parameter("fsh", [BSH, D], F32, isOutput=False)
    pidx_ext = nc.declare_dram_parameter("pidx", [128, 16], mybir.dt.int32, isOutput=False)
    sexp_ext = nc.declare_dram_parameter("sexp", [128, NBLK * NCH], F32, isOutput=True)
    pdot_ext = nc.declare_dram_parameter("pdot", [128, 16], F32, isOutput=True)
    rf_ext = nc.declare_dram_parameter("rf30", [128, NBLK], F32, isOutput=True)
    rw_ext = nc.declare_dram_parameter("rwrec", [128, CT], F32, isOutput=True)

    mult = mybir.AluOpType.mult
    AF = mybir.ActivationFunctionType

    with ExitStack() as ctx:
        tc = ctx.enter_context(tile.TileContext(nc))
        const = ctx.enter_context(tc.tile_pool(name="const", bufs=1))
        resident = ctx.enter_context(tc.tile_pool(name="resident", bufs=1))
        fstage = ctx.enter_context(tc.tile_pool(name="fstage", bufs=4))
        wstage = ctx.enter_context(tc.tile_pool(name="wstage", bufs=4))
        wbfp = ctx.enter_context(tc.tile_pool(name="wbfp", bufs=3))
        scrp = ctx.enter_context(tc.tile_pool(name="scrp", bufs=3))
        esp = ctx.enter_context(tc.tile_pool(name="esp", bufs=4))
        ptr_pool = ctx.enter_context(tc.tile_pool(name="ptr", bufs=1, space="PSUM"))
        pmmA = ctx.enter_context(tc.tile_pool(name="pmmA", bufs=2, space="PSUM"))
        pmmB = ctx.enter_context(tc.tile_pool(name="pmmB", bufs=1, space="PSUM"))
        pmmC = ctx.enter_context(tc.tile_pool(name="pmmC", bufs=1, space="PSUM"))

        id_bf = const.tile([128, 128], BF16)
        make_identity(nc, id_bf[:])
        zbias = const.tile([128, 1], F32)
        nc.vector.memset(zbias[:], 0.0)
        nbias = const.tile([128, 1], F32)
        nc.vector.memset(nbias[:], -S)

        # resident tensors
        wT = resident.tile([128, KC, CSHP], FP8)      # normalized w, transposed
        fT = resident.tile([128, KC, B], FP8)        # raw f, transposed
        G = resident.tile([128, 16, D], F32)         # gathered positive w rows
        fsh_t = resident.tile([128, 2, D], F32)      # this core's f rows (raw)
        sexp_t = resident.tile([128, NBLK * NCH], F32)
        pdot_t = resident.tile([128, 16], F32)
        ss_f = resident.tile([128, NBLK], F32)
        tmp_f = resident.tile([128, NBLK], F32)
        rf30 = resident.tile([128, NBLK], F32)
        rf30s = resident.tile([128, NBLK], F32)
        ss_w = resident.tile([128, CT], F32)
        tmp_w = resident.tile([128, CT], F32)
        rw_rec = resident.tile([128, CT], F32)
        pidx_t = resident.tile([128, 16], mybir.dt.int32)

        # ---- positives gather (early: overlaps with everything) ----
        nc.sync.dma_start(pidx_t[:], pidx_ext[:, :])
        nc.sync.dma_start(fsh_t[:, 0, :], fsh_ext[0:128, :])
        nc.sync.dma_start(fsh_t[:, 1, :], fsh_ext[128:256, :])
        for s in range(16):
            nc.gpsimd.indirect_dma_start(
                out=G[:, s, :],
                out_offset=None,
                in_=w_ext[:, :],
                in_offset=bass.IndirectOffsetOnAxis(ap=pidx_t[:, s : s + 1], axis=0),
            )

        # ---- w path: row norms, scale to unit rows (bf16), transpose ----
        nc.vector.memset(rw_rec[:], 0.0)  # tail tile covers <128 partitions
        nc.vector.memset(sexp_t[:], 0.0)  # strip-exp fills col m*NCH only
        for ct in range(CT):
            r0 = ct * 128
            wt = wstage.tile([128, D], F32, tag="w")
            nc.sync.dma_start(wt[:, :], wsh_ext[r0 : r0 + 128, :])
            sc = scrp.tile([128, D], F32, tag="scr")
            nc.scalar.activation(
                sc[:], wt[:], AF.Square,
                bias=zbias[:], scale=1.0,
                accum_out=ss_w[:, ct : ct + 1],
            )
            # zero pad rows: keep sqrt/recip finite (wT pad cols end up 0)
            nc.vector.tensor_scalar_max(
                ss_w[:, ct : ct + 1], ss_w[:, ct : ct + 1], 1e-12
            )
            nc.scalar.activation(
                tmp_w[:, ct : ct + 1], ss_w[:, ct : ct + 1],
                AF.Sqrt, bias=zbias[:], scale=1.0,
            )
            nc.vector.reciprocal(
                rw_rec[:, ct : ct + 1], tmp_w[:, ct : ct + 1]
            )
            wbf = wbfp.tile([128, D], BF16, tag="wbf")
            nc.vector.tensor_scalar_mul(
                wbf[:, :], wt[:, :], rw_rec[:, ct : ct + 1]
            )
            pt = ptr_pool.tile([128, KC, 128], BF16, tag="ptr")
            for k in range(KC):
                nc.tensor.transpose(
                    pt[:, k, :], wbf[:, k * 128 : (k + 1) * 128], id_bf[:]
                )
            nc.vector.tensor_scalar_mul(
                wT[:, :, r0 : r0 + 128], pt[:], F8S
            )
        nc.sync.dma_start(rw_ext[:, :], rw_rec[:])

        # ---- f path: row norms (for ACT scale), raw transpose ----
        for m in range(NBLK):
            ft = fstage.tile([128, D], F32, tag="f")
            nc.sync.dma_start(ft[:], f_ext[m * 128 : (m + 1) * 128, :])
            sc = scrp.tile([128, D], F32, tag="scr")
            nc.scalar.activation(
                sc[:], ft[:], AF.Square,
                bias=zbias[:], scale=1.0,
                accum_out=ss_f[:, m : m + 1],
            )
            fb = wbfp.tile([128, D], BF16, tag="fb")
            nc.vector.tensor_copy(out=fb[:], in_=ft[:])
            pt = ptr_pool.tile([128, KC, 128], BF16, tag="ptr")
            for k in range(KC):
                nc.tensor.transpose(
                    pt[:, k, :], fb[:, k * 128 : (k + 1) * 128], id_bf[:]
                )
            nc.vector.tensor_scalar_mul(
                fT[:, :, m * 128 : (m + 1) * 128], pt[:], F8S
            )
        # rf30 = 30 / |f|  (sqrt(ss/900) then reciprocal)
        nc.scalar.activation(
            tmp_f[:], ss_f[:], AF.Sqrt, bias=zbias[:], scale=1.0 / (S * S)
        )
        nc.vector.reciprocal(rf30[:], tmp_f[:])
        nc.vector.tensor_scalar_mul(rf30s[:], rf30[:], 1.0 / (F8S * F8S))
        nc.sync.dma_start(rf_ext[:, :], rf30[:])


        # ---- positive dots: pdot[p, j+8h] = f[h*128+p] . G[p, j+8h] ----
        for h in range(2):
            for j in range(LMAX):
                s = j + LMAX * h
                sc = scrp.tile([128, D], F32, tag="scr")
                nc.vector.scalar_tensor_tensor(
                    out=sc[:], in0=G[:, s, :], scalar=1.0, in1=fsh_t[:, h, :],
                    op0=mult, op1=mult,
                    accum_out=pdot_t[:, s : s + 1],
                )
        nc.sync.dma_start(pdot_ext[:, :], pdot_t[:])

        # ---- main loop: dots -> exp(30*cos - 30) -> per-row accumulate ----
        for m in range(NBLK):
            pA = pmmA.tile([128, 2, NW], F32, tag="mmA", name=f"mmA_{m}")
            pB = pmmB.tile([128, 2, NW], F32, tag="mmB", name=f"mmB_{m}")
            pC = pmmC.tile([128, NW], F32, tag="mmC", name=f"mmC_{m}")
            # single-buffered B and C run first so their evicts have a full
            # block of slack before block m+1 reuses the banks; A (double
            # buffered) never stalls.
            ps = [pB[:, 0, :], pB[:, 1, :], pC[:], pA[:, 0, :], pA[:, 1, :]]
            for k2 in range(KC // 2):
                for i, n in enumerate((2, 3, 4, 0, 1)):
                    nc.tensor.matmul(
                        ps[i],
                        fT[:, 2 * k2 : 2 * k2 + 2, m * 128 : (m + 1) * 128],
                        wT[:, 2 * k2 : 2 * k2 + 2, n * NW : (n + 1) * NW],
                        start=(k2 == 0),
                        stop=(k2 == KC // 2 - 1),
                        perf_mode=mybir.MatmulPerfMode.DoubleRow,
                    )
            strip = esp.tile([128, NCH * NW], F32, tag="es", name=f"st{m}")
            nc.any.tensor_copy(out=strip[:, 2 * NW : 4 * NW], in_=pB[:])
            nc.any.tensor_copy(out=strip[:, 4 * NW : 5 * NW], in_=pC[:])
            nc.any.tensor_copy(out=strip[:, 0 : 2 * NW], in_=pA[:])
            edump = esp.tile([128, NCH * NW], BF16, tag="ed", name=f"ed{m}")
            nc.scalar.activation(
                edump[:], strip[:], AF.Exp,
                bias=nbias[:], scale=rf30s[:, m : m + 1],
                accum_out=sexp_t[:, m * NCH : m * NCH + 1],
            )
        nc.sync.dma_start(sexp_ext[:, :], sexp_t[:])


    nc.finalize()
    return nc


def _get_graph():
    global _GRAPH
    if _GRAPH is None:
        _GRAPH = build_graph()
    return _GRAPH


def make_in_maps(f, lab_word2vec, lab_pinds):
    f = np.ascontiguousarray(np.asarray(f, dtype=np.float32))
    w = np.ascontiguousarray(np.asarray(lab_word2vec, dtype=np.float32))
    pinds = np.asarray(lab_pinds, dtype=np.int64)
    in_maps = []
    for i in range(NCORES):
        # slot s = j + 8h at partition p  <-  lab_pinds[i*256 + h*128 + p, j]
        pidx = np.zeros((128, 16), dtype=np.int32)
        for h in range(2):
            for j in range(LMAX):
                pidx[:, j + LMAX * h] = pinds[
                    i * BSH + h * 128 : i * BSH + h * 128 + 128, j
                ]
        wsh = np.zeros((CSHP, D), dtype=np.float32)
        wsh[:CSH] = w[i * CSH : (i + 1) * CSH]
        in_maps.append(
            {
                "f": f,
                "wsh": wsh,
                "w": w,
                "fsh": np.ascontiguousarray(f[i * BSH : (i + 1) * BSH]),
                "pidx": pidx,
            }
        )
    return in_maps


def combine(outs, lab_pinds, lengths):
    """outs: list of 8 dicts with sexp/pdot/rf30/rwrec. Returns float32 loss."""
    pinds = np.asarray(lab_pinds, dtype=np.int64)
    lens = np.asarray(lengths, dtype=np.int64)

    # S_shift[b] = sum_c exp(30 cos - 30)
    s_shift = np.zeros(B, dtype=np.float64)
    for i in range(NCORES):
        se = outs[i]["sexp"].astype(np.float64)  # [128, NBLK*NCH]
        per_block = se.reshape(128, NBLK, NCH).sum(axis=2)  # [128, NBLK]
        s_shift += per_block.T.reshape(B)  # b = m*128 + p
    # the 60 zero-pad classes per core contribute exp(-30) each (cos = 0)
    s_shift -= NCORES * (CSHP - CSH) * math.exp(-S)

    rf = outs[0]["rf30"].astype(np.float64).T.reshape(B) / S  # 1/|f_b|

    rw = np.zeros(C, dtype=np.float64)
    for i in range(NCORES):
        rr = outs[i]["rwrec"].astype(np.float64)  # [128, CT]
        rw[i * CSH : (i + 1) * CSH] = rr.T.reshape(CSHP)[:CSH]

    # positive raw dots -> [B, LMAX]
    pdot = np.zeros((B, LMAX), dtype=np.float64)
    for i in range(NCORES):
        pd = outs[i]["pdot"].astype(np.float64)  # [128, 16]
        for h in range(2):
            for j in range(LMAX):
                pdot[i * BSH + h * 128 : i * BSH + h * 128 + 128, j] = pd[
                    :, j + LMAX * h
                ]

    cos = pdot * rf[:, None] * rw[pinds]  # [B, LMAX]
    cos_m, sin_m = math.cos(M_MARGIN), math.sin(M_MARGIN)
    th = math.cos(math.pi - M_MARGIN)
    mm = math.sin(math.pi - M_MARGIN) * M_MARGIN
    sine = np.sqrt(np.clip(1.0 - cos * cos, 0.0, 1.0))
    phi = cos * cos_m - sine * sin_m
    phi = np.where(cos > th, phi, cos - mm)

    mask = (np.arange(LMAX)[None, :] < lens[:, None]).astype(np.float64)
    corr = (mask * (np.exp(S * phi - S) - np.exp(S * cos - S))).sum(axis=1)
    z = S + np.log(s_shift + corr)  # logsumexp of outputs, [B]
    pos_sum = (mask * (S * phi)).sum(axis=1)
    L = lens.astype(np.float64)
    per_sample = (L * z - pos_sum) / (L * L)
    return np.float32(per_sample.mean())


def kernel(f, labels, lab_word2vec, lab_pinds, lengths):
    nc = _get_graph()
    in_maps = make_in_maps(f, lab_word2vec, lab_pinds)
    res = run_bass_kernel_spmd(nc, in_maps, core_ids=list(range(NCORES)))
    return combine(res.results, lab_pinds, lengths)



# revision 2
# speedup vs baseline: 1.8485x; 1.8485x over previous
"""ArcFace combined-margin loss kernel for 8 TRN2 NeuronCores.

Strategy
--------
reference: cos = (f @ w.T) / (|f||w|); phi = arcface(cos);
outputs = s*(labels*phi + (1-labels)*cos); loss = mean over rows of
-(sum of log_softmax(outputs) at lab_pinds, masked) / L^2.

labels is the multi-hot of (lab_pinds, lengths), so outputs differs from
s*cos only at <=8 entries/row.  The only device-scale compute is the
dense denominator  sexp[b] = sum_c exp(30*cos[b,c] - 30)  (B*C*D MACs +
B*C exps).  Everything else is O(B*L*D + C*D) and runs on host float64.

Device (per core, C-sharded: 2500 classes/core zero-padded to 2560):
  inputs are HOST-prepared fp8 operands, pre-normalized, pre-scaled and
  pre-transposed so the exp argument has a CONSTANT scale/bias:
     fT8[d, b] = fp8(30 * f[b,d] / |f_b|)      [512, 2048]
     wT8[d, c] = fp8(16 * w[c,d] / |w_c|)      [512, 2560]  (class shard)
  dot_psum = sum_d fT8*wT8 = 480*cos, so exp arg = dot/16 - 30 for every
  element -- ACT instructions need no per-row scale and can span any
  PSUM bank group.  Loop: 80 (row-block m, class-chunk n) tiles in
  block-major order; each tile = 2 fp8 DoubleRow matmuls (K=256) into
  one PSUM bank; groups of 4 tiles (4 banks, double-buffered 4+4) are
  evicted by one ACT Exp -> bf16 SBUF strip; DVE tensor_reduce sums each
  block's 2560-wide strip into sexp[128, 16].
Host (numpy float64): positive dots f.w[pinds] exactly, arcface margin,
denominator correction at positives, logsumexp, masked ragged CE, mean.
No collectives (the only cross-core reduction is summing 8 sexp
partials on host during unsharding).
"""

import math
import sys

import numpy as np
import ml_dtypes

for _p in ("/opt/trn_rl_repo",):
    if _p not in sys.path:
        sys.path.append(_p)

import concourse.bass as bass
import concourse.bacc as bacc
import concourse.mybir as mybir
import concourse.tile as tile
from concourse.bass_utils import run_bass_kernel_spmd
from contextlib import ExitStack

B, C, D, LMAX = 2048, 20000, 512, 8
NCORES = 8
CSH = C // NCORES          # 2500 real classes per core
CSHP = 2560                # padded to 5*512 (bank-aligned chunks)
NBLK = B // 128            # 16 row blocks
NW = 512                   # matmul N-chunk width (exactly one PSUM bank)
NCH = CSHP // NW           # 5 chunks per core
KC = D // 128              # 4 contraction chunks (128 partitions each)
NT = NBLK * NCH            # 80 (m, n) tiles
NG = NT // 4               # 20 ACT eviction groups of 4 banks
S = 30.0
M_MARGIN = 0.5
FSC = 30.0                 # f rows scaled to 30*unit
WSC = 16.0                 # w rows scaled to 16*unit
# psum dot = FSC*WSC*cos; exp arg = dot/WSC - 30 = 30*cos - 30

F32 = mybir.dt.float32
BF16 = mybir.dt.bfloat16
FP8 = mybir.dt.float8e4
E4M3 = ml_dtypes.float8_e4m3

_GRAPH = None


def build_graph():
    nc = bacc.Bacc()
    fT_ext = nc.declare_dram_parameter("fT8", [D, B], FP8, isOutput=False)
    wT_ext = nc.declare_dram_parameter("wT8", [D, CSHP], FP8, isOutput=False)
    sexp_ext = nc.declare_dram_parameter("sexp", [128, NBLK], F32, isOutput=True)

    AF = mybir.ActivationFunctionType

    with ExitStack() as ctx:
        tc = ctx.enter_context(tile.TileContext(nc))
        const = ctx.enter_context(tc.tile_pool(name="const", bufs=1))
        resident = ctx.enter_context(tc.tile_pool(name="resident", bufs=1))
        pmm = ctx.enter_context(tc.tile_pool(name="pmm", bufs=2, space="PSUM"))

        nbias = const.tile([128, 1], F32)
        nc.vector.memset(nbias[:], -S)

        fT = resident.tile([128, KC, B], FP8)
        wT = resident.tile([128, KC, CSHP], FP8)
        strip = resident.tile([128, NT, NW], BF16)
        sexp_t = resident.tile([128, NBLK], F32)

        # DMA order: first tiles need wT chunks 0-3 + fT block-group 0.
        for n in range(4):
            for k in range(KC):
                nc.sync.dma_start(
                    wT[:, k, n * NW : (n + 1) * NW],
                    wT_ext[k * 128 : (k + 1) * 128, n * NW : (n + 1) * NW],
                )
        for k in range(KC):
            nc.sync.dma_start(
                fT[:, k, 0:NW], fT_ext[k * 128 : (k + 1) * 128, 0:NW]
            )
        for k in range(KC):
            nc.sync.dma_start(
                wT[:, k, 4 * NW : 5 * NW],
                wT_ext[k * 128 : (k + 1) * 128, 4 * NW : 5 * NW],
            )
        for g in range(1, 4):
            for k in range(KC):
                nc.sync.dma_start(
                    fT[:, k, g * NW : (g + 1) * NW],
                    fT_ext[k * 128 : (k + 1) * 128, g * NW : (g + 1) * NW],
                )

        # main loop: tiles t = 5*m + n in block-major order; 4 banks/group
        for g in range(NG):
            P = pmm.tile([128, 4, NW], F32, tag="mm", name=f"mm{g}")
            for j in range(4):
                t = 4 * g + j
                m, n = t // NCH, t % NCH
                for k2 in range(KC // 2):
                    nc.tensor.matmul(
                        P[:, j, :],
                        fT[:, 2 * k2 : 2 * k2 + 2, m * 128 : (m + 1) * 128],
                        wT[:, 2 * k2 : 2 * k2 + 2, n * NW : (n + 1) * NW],
                        start=(k2 == 0),
                        stop=(k2 == KC // 2 - 1),
                        perf_mode=mybir.MatmulPerfMode.DoubleRow,
                    )
            nc.scalar.activation(
                strip[:, 4 * g : 4 * g + 4, :], P[:], AF.Exp,
                bias=nbias[:], scale=1.0 / WSC,
            )
            # blocks fully evicted by this group get their row-sum on DVE
            for m in range(NBLK):
                if (NCH * m + NCH - 1) // 4 == g:
                    nc.vector.tensor_reduce(
                        sexp_t[:, m : m + 1],
                        strip[:, NCH * m : NCH * m + NCH, :],
                        axis=mybir.AxisListType.XY,
                        op=mybir.AluOpType.add,
                    )
        nc.sync.dma_start(sexp_ext[:, :], sexp_t[:])

    nc.finalize()
    return nc


def _get_graph():
    global _GRAPH
    if _GRAPH is None:
        _GRAPH = build_graph()
    return _GRAPH


def make_in_maps(f, lab_word2vec, lab_pinds=None):
    f = np.asarray(f, dtype=np.float32)
    w = np.asarray(lab_word2vec, dtype=np.float32)
    fn = np.sqrt((f.astype(np.float64) ** 2).sum(axis=1))
    wn = np.sqrt((w.astype(np.float64) ** 2).sum(axis=1))
    fT8 = np.ascontiguousarray(
        (f * (FSC / fn)[:, None].astype(np.float32)).T
    ).astype(E4M3)
    w8 = (w * (WSC / wn)[:, None].astype(np.float32)).astype(E4M3)
    in_maps = []
    for i in range(NCORES):
        wT8 = np.zeros((D, CSHP), dtype=E4M3)
        wT8[:, :CSH] = w8[i * CSH : (i + 1) * CSH].T
        in_maps.append({"fT8": fT8, "wT8": wT8})
    return in_maps


def combine(outs, f, lab_word2vec, lab_pinds, lengths):
    """outs: list of 8 dicts with sexp [128, NBLK]. Returns float32 loss."""
    f = np.asarray(f, dtype=np.float64)
    w = np.asarray(lab_word2vec, dtype=np.float64)
    pinds = np.asarray(lab_pinds, dtype=np.int64)
    lens = np.asarray(lengths, dtype=np.int64)

    # s_shift[b] = sum_c exp(30 cos - 30); b = m*128 + p
    s_shift = np.zeros(B, dtype=np.float64)
    for i in range(NCORES):
        s_shift += outs[i]["sexp"].astype(np.float64).T.reshape(B)
    # the 60 zero-pad classes per core contribute exp(-30) each (cos = 0)
    s_shift -= NCORES * (CSHP - CSH) * math.exp(-S)

    fn = np.sqrt((f * f).sum(axis=1))     # [B]
    wn = np.sqrt((w * w).sum(axis=1))     # [C]
    pd = np.einsum("bjd,bd->bj", w[pinds], f)              # [B, LMAX]
    cos = pd / np.maximum(fn[:, None] * wn[pinds], 1e-8)

    cos_m, sin_m = math.cos(M_MARGIN), math.sin(M_MARGIN)
    th = math.cos(math.pi - M_MARGIN)
    mm = math.sin(math.pi - M_MARGIN) * M_MARGIN
    sine = np.sqrt(np.clip(1.0 - cos * cos, 0.0, 1.0))
    phi = cos * cos_m - sine * sin_m
    phi = np.where(cos > th, phi, cos - mm)

    mask = (np.arange(LMAX)[None, :] < lens[:, None]).astype(np.float64)
    corr = (mask * (np.exp(S * phi - S) - np.exp(S * cos - S))).sum(axis=1)
    z = S + np.log(s_shift + corr)  # logsumexp of outputs, [B]
    pos_sum = (mask * (S * phi)).sum(axis=1)
    L = lens.astype(np.float64)
    per_sample = (L * z - pos_sum) / (L * L)
    return np.float32(per_sample.mean())


def kernel(f, labels, lab_word2vec, lab_pinds, lengths):
    nc = _get_graph()
    in_maps = make_in_maps(f, lab_word2vec)
    res = run_bass_kernel_spmd(nc, in_maps, core_ids=list(range(NCORES)))
    return combine(res.results, f, lab_word2vec, lab_pinds, lengths)
